# revision 26
# baseline (speedup 1.0000x reference)
"""Trainium2 Bass kernel for nn_CustomLoss_62921270887106.

Loss = BCE(class_pred, class_gt) (mean, torch log-clamp at -100)
     + mean_b( 0.5 * sum_jc[ (class_pred>=0.5) * (reg_pred-reg_gt)^2 ] / (1 + sum_j class_gt) )

Strategy: pure data parallel over the batch dim on 8 NeuronCores.
Each core reduces its 125000-sample shard to per-partition partial sums
[128, 2] (col0: sum of BCE log-terms, col1: sum of 0.5*sq/nj); the host
sums the 8x128 partials in float64 and combines.

v2 (this file): the baseline was bound by single-queue DMA (~700ns
inter-transfer gap per DMA on qSyncDynamicHW -> DMA engines only 80%
busy) plus gpsimd/DVE at ~7.7/8.1us per 7.85us tile. Changes:
  - DMAs alternate between the two HWDGE queues (sync + scalar) by
    pipeline step parity, hiding per-instruction queue overhead.
  - ACT computes a dense sgn = Sign(p - 0.5) so the DVE mask op reads
    dense instead of stride-3 (stt 2.98us -> ~1.25us).
  - The full-width sub is split gpsimd[0:2805] / DVE[2805:3111] to
    balance both engines at ~7.3us/tile.
  - Software-pipelined emission: step i issues DMA(i), subs(i-2),
    ACT ops(i-3), DVE mask-chain(i-4), so no engine ever waits on a
    same-step cross-engine dependency.
  - 1/(2*(1+nj)) computed in batched slabs on ACT (Ln/Exp), final
    weighted sum via one DVE tensor_tensor_reduce.
  - Last main tile split into 4 small slices so the drain after the
    last DMA is ~4us instead of ~14us.
"""

import sys

for _p in ("/opt/trn_rl_repo",):
    if _p not in sys.path:
        sys.path.insert(0, _p)

import numpy as np

import concourse.bass as bass
import concourse.tile as tile
from concourse import bacc, mybir
from concourse.bass_utils import run_bass_kernel_spmd

F32 = mybir.dt.float32
BF16 = mybir.dt.bfloat16
AF = mybir.ActivationFunctionType
ALU = mybir.AluOpType
AX = mybir.AxisListType

B = 1_000_000
J = 17
C = 3
N_CORES = 8
N_LOC = B // N_CORES            # 125000 samples per core
P = 128
K = 61                          # samples per partition per main tile
M = J * C                       # 51 floats per sample
SPLIT = 2816                    # gpsimd sub [0:2816], DVE [2816:3111] (64B aligned)
DMA_SINGLE_QUEUE = False        # debug: route all DMAs through sync
USE_DENSE_MASK = True           # ACT Copy p -> dense, stt reads dense
USE_TTR_EPILOGUE = False        # debug: tensor_tensor_reduce epilogue

_PROGRAM_CACHE = {}


def _build_program(n_loc=N_LOC):
    TILE_SAMPLES = P * K             # 7808
    NT_MAIN = n_loc // TILE_SAMPLES  # 16
    MAIN = NT_MAIN * TILE_SAMPLES
    TAIL = n_loc - MAIN              # 72

    nc = bacc.Bacc("TRN2", target_bir_lowering=False, debug=False,
                   num_devices=N_CORES)

    o_dram = nc.dram_tensor("output", [n_loc, J, C], F32, kind="ExternalInput").ap()
    t_dram = nc.dram_tensor("target", [n_loc, J, C], F32, kind="ExternalInput").ap()
    partials = nc.dram_tensor("partials", [P, 2], F32, kind="ExternalOutput").ap()

    o_flat = o_dram.rearrange("b j c -> b (j c)")
    t_flat = t_dram.rearrange("b j c -> b (j c)")
    o_main = o_flat[0:MAIN, :].rearrange("(n p k) m -> n p (k m)", p=P, k=K)
    t_main = t_flat[0:MAIN, :].rearrange("(n p k) m -> n p (k m)", p=P, k=K)
    o_tail = o_flat[MAIN:n_loc, :]   # [72, 51]
    t_tail = t_flat[MAIN:n_loc, :]

    # Pipeline step list: tail first (hides under ramp), then full main
    # tiles, then the last main tile as 4 small slices (short drain).
    # Each entry: (o_src, t_src, rows, k, sq/nj col offset, bce col).
    steps = []
    cols = 0
    bcol = 0
    steps.append(("tail", None, TAIL, 1, None, None))
    for t in range(NT_MAIN - 1):
        steps.append(("main", t, P, K, None, None))
    LASTK = (16, 15, 15, 15)
    assert sum(LASTK) == K
    off = 0
    for k in LASTK:
        steps.append(("slice", off, P, k, None, None))
        off += k
    # assign column offsets in emission order
    fixed = []
    for kind, idx, rows, k, _, _ in steps:
        fixed.append((kind, idx, rows, k, cols, bcol))
        cols += k
        bcol += 1
    steps = fixed
    NCOLS = cols                    # 977
    NBCE = bcol                     # 20
    TAILCOL = 0                     # tail writes col 0 / bce col 0
    NSTEP = len(steps)

    with tile.TileContext(nc) as tc:
        with (
            tc.tile_pool(name="inp", bufs=4) as inp,
            tc.tile_pool(name="work", bufs=2) as work,
            tc.tile_pool(name="persist", bufs=1) as persist,
        ):
            sqbuf = persist.tile([P, NCOLS], F32)
            njbuf = persist.tile([P, NCOLS], F32)
            rnjbuf = persist.tile([P, NCOLS], F32)
            bcecols = persist.tile([P, NBCE], F32)
            outtile = persist.tile([P, 2], F32)
            junk = persist.tile([P, NCOLS], F32)

            # tail columns are only written for rows < TAIL; zero them
            nc.gpsimd.memset(sqbuf[:, TAILCOL:TAILCOL + 1], 0.0)
            nc.gpsimd.memset(njbuf[:, TAILCOL:TAILCOL + 1], 0.0)
            nc.gpsimd.memset(bcecols[:, TAILCOL:TAILCOL + 1], 0.0)

            # per-step state handed between pipeline stages
            st = [dict() for _ in range(NSTEP)]

            def src_aps(i):
                kind, idx, rows, k, c0, bc = steps[i]
                if kind == "tail":
                    return o_tail, t_tail
                if kind == "main":
                    return o_main[idx], t_main[idx]
                # slice of the last main tile: columns of sample-range
                t = NT_MAIN - 1
                a, b = idx * M, (idx + k) * M
                return o_main[t][:, a:b], t_main[t][:, a:b]

            def stage_dma(i):
                kind, idx, rows, k, c0, bc = steps[i]
                o_src, t_src = src_aps(i)
                # without the dense mask, "to" lives until the mask stage
                # (i-4) reads p_flat from it, needing one extra buf
                to = inp.tile([P, K * M], F32, tag="to",
                              bufs=4 if USE_DENSE_MASK else 5)
                tt = inp.tile([P, K * M], F32, tag="tt", bufs=4)
                eng = nc.sync if (i % 2 == 0 or DMA_SINGLE_QUEUE) else nc.scalar
                eng.dma_start(out=to[:rows, 0:k * M], in_=o_src)
                eng.dma_start(out=tt[:rows, 0:k * M], in_=t_src)
                st[i]["to"] = to
                st[i]["tt"] = tt

            def stage_sub(i):
                kind, idx, rows, k, c0, bc = steps[i]
                to, tt = st[i]["to"], st[i]["tt"]
                # full-width sub stays on gpsimd: a DVE split fights gpsimd
                # for the shared SBUF write port and loses ~3-6x throughput
                dfull = work.tile([P, K * M], F32, tag="dfull")
                w = k * M
                nc.gpsimd.tensor_sub(dfull[:rows, 0:w],
                                     to[:rows, 0:w], tt[:rows, 0:w])
                # dense mask source: sgn = Sign(p - 0.5)
                o4 = to[:rows, 0:w].rearrange("p (k j c) -> p k j c",
                                              k=k, j=J, c=C)
                p_flat = o4[:, :, :, 2].rearrange("p k j -> p (k j)")
                if USE_DENSE_MASK:
                    pcp = work.tile([P, K * J], BF16, tag="pcp", bufs=3)
                    nc.scalar.activation(pcp[:rows, 0:k * J], p_flat, AF.Copy)
                    st[i]["pcp"] = pcp
                else:
                    st[i]["p_flat"] = p_flat
                # nj = sum_J g  (strided read of class col of target)
                t4 = tt[:rows, 0:w].rearrange("p (k j c) -> p k j c",
                                              k=k, j=J, c=C)
                nc.vector.tensor_reduce(njbuf[:rows, c0:c0 + k],
                                        t4[:, :, :, 2], axis=AX.X, op=ALU.add)
                st[i]["dfull"] = dfull

            def stage_act(i):
                kind, idx, rows, k, c0, bc = steps[i]
                dfull = st[i]["dfull"]
                w = k * M
                d4 = dfull[:rows, 0:w].rearrange("p (k j c) -> p k j c",
                                                 k=k, j=J, c=C)
                dc = d4[:, :, :, 2].rearrange("p k j -> p (k j)")
                # BCE: a = |dc|*(1-2^-23); L = ln(1 - a), accum -> bce col
                tabs = work.tile([P, K * J], F32, tag="tabs")
                nc.scalar.activation(tabs[:rows, 0:k * J], dc, AF.Abs,
                                     scale=float(1.0 - 2.0 ** -23))
                nc.scalar.activation(tabs[:rows, 0:k * J],
                                     tabs[:rows, 0:k * J], AF.Ln,
                                     bias=1.0, scale=-1.0,
                                     accum_out=bcecols[:rows, bc:bc + 1])
                d2 = work.tile([P, K * J * 2], BF16, tag="d2")
                d2v = d2[:rows, 0:k * J * 2].rearrange(
                    "p (k j c) -> p k j c", k=k, j=J, c=2)
                nc.scalar.activation(d2v, d4[:, :, :, 0:2], AF.Square)
                st[i]["d2"] = d2

            def stage_mask(i):
                kind, idx, rows, k, c0, bc = steps[i]
                d2 = st[i]["d2"]
                e = work.tile([P, K * J], BF16, tag="e")
                with nc.allow_low_precision("pair-sums <= 2, bf16 ok"):
                    nc.vector.tensor_reduce(
                        e[:rows, 0:k * J],
                        d2[:rows, 0:k * J * 2].rearrange("p (a c) -> p a c", c=2),
                        axis=AX.X, op=ALU.add)
                in0 = (st[i]["pcp"][:rows, 0:k * J] if USE_DENSE_MASK
                       else st[i]["p_flat"])
                nc.vector.scalar_tensor_tensor(
                    out=e[:rows, 0:k * J], in0=in0,
                    scalar=0.5, in1=e[:rows, 0:k * J],
                    op0=ALU.is_ge, op1=ALU.mult)
                nc.vector.tensor_reduce(
                    sqbuf[:rows, c0:c0 + k],
                    e[:rows, 0:k * J].rearrange("p (k j) -> p k j", k=k),
                    axis=AX.X, op=ALU.add)
                st[i].clear()

            # software pipeline: dma(i), sub(i-2), act(i-3), mask(i-4)
            D_SUB, D_ACT, D_MASK = 2, 3, 4

            for i in range(NSTEP + D_MASK):
                if i < NSTEP:
                    stage_dma(i)
                if 0 <= i - D_SUB < NSTEP:
                    stage_sub(i - D_SUB)
                if 0 <= i - D_ACT < NSTEP:
                    stage_act(i - D_ACT)
                if 0 <= i - D_MASK < NSTEP:
                    stage_mask(i - D_MASK)

            # epilogue: rnj = 1/(1+nj) = exp(-ln(1+nj)) on ACT (idle in the
            # drain; DVE reciprocal measured 7.5us for this slab), host
            # folds the 0.5; then wsum = sum sq*rnj ; bce = sum bcecols
            nc.scalar.activation(rnjbuf[:], njbuf[:], AF.Ln, bias=1.0)
            nc.scalar.activation(rnjbuf[:], rnjbuf[:], AF.Exp, scale=-1.0)
            if USE_TTR_EPILOGUE:
                nc.vector.tensor_tensor_reduce(
                    out=junk[:], in0=sqbuf[:], in1=rnjbuf[:], scale=1.0,
                    scalar=0.0, op0=ALU.mult, op1=ALU.add,
                    accum_out=outtile[:, 1:2])
            else:
                nc.vector.tensor_mul(junk[:], sqbuf[:], rnjbuf[:])
                nc.vector.tensor_reduce(outtile[:, 1:2], junk[:],
                                        axis=AX.X, op=ALU.add)
            nc.vector.tensor_reduce(outtile[:, 0:1], bcecols[:], axis=AX.X,
                                    op=ALU.add)
            nc.sync.dma_start(out=partials, in_=outtile[:])

    nc.compile()
    return nc


def _get_program(n_loc=N_LOC):
    if n_loc not in _PROGRAM_CACHE:
        _PROGRAM_CACHE[n_loc] = _build_program(n_loc)
    return _PROGRAM_CACHE[n_loc]


def _run_shards(output, target, trace=False, **kw):
    nc = _get_program()
    o = np.ascontiguousarray(np.asarray(output, dtype=np.float32))
    t = np.ascontiguousarray(np.asarray(target, dtype=np.float32))
    in_maps = []
    for i in range(N_CORES):
        sl = slice(i * N_LOC, (i + 1) * N_LOC)
        in_maps.append({"output": o[sl], "target": t[sl]})
    return run_bass_kernel_spmd(nc, in_maps, list(range(N_CORES)),
                                trace=trace, **kw)


def _combine(results):
    bce_sum = 0.0
    wsq_sum = 0.0
    for r in results:
        p = np.asarray(r["partials"], dtype=np.float64)
        bce_sum += p[:, 0].sum()
        wsq_sum += p[:, 1].sum()
    loss = -bce_sum / (B * J) + 0.5 * wsq_sum / B
    return np.float32(loss)


def kernel(output, target):
    res = _run_shards(output, target, trace=False)
    return _combine(res.results)
